# revision 43
# baseline (speedup 1.0000x reference)
"""Trainium2 Bass kernel for nn_Decoder (dense MLP).

Computes out = relu(V @ W1 + b1) @ W2 + b2 for V [262144, 1024],
W1 [1024, 128], W2 [128, 4].

Strategy
--------
Data-parallel over 8 NeuronCores: V is sharded along rows (32768 rows per
core); the small weights are replicated. Each core's V shard is
host-prepacked to [group, partition, k-chunk, row] so the contraction dim
lands on SBUF partitions and every 2 MiB group DMA is 128 descriptors of
16 KB fully-sequential HBM reads — no on-chip transposes.

Per core, the kernel computes h.T = W1.T @ V.T via PSUM-accumulated
matmuls over 8 K-chunks (lhsT = the natural W1 layout), applies
ReLU(+b1) on the scalar engine reading PSUM (emitting f16 h), then
out.T = W2.T @ h.T as a single f16 matmul, adds b2 on the vector
engine, and stores out.T [4, 32768] contiguously. The host transposes
the gathered outputs back.

V streams as fp8 e3m4 (4 mantissa bits — fp8 e4m3 fails the accuracy
gate, e3m4 passes with ~1.4e-2 vs the 2e-2 budget): 32 MiB/core of HBM
traffic against the ~358 GB/s per-core ceiling. The PE consumes the
e3m4 moving operand against f16 stationary weights at 1 col/cycle with
a full-precision upconvert (double-pumped fp8 would truncate to e6m3
and fail the gate — deliberately avoided).

Pipeline structure (keeps the PE at its 213 ns/matmul streaming rate):
  - ReLU of chunk-pair p-1 is emitted AFTER pair p's eight L1 matmuls,
    so the cross-engine ReLU/PSUM-drain latency hides under a full pair
    of L1 compute instead of stalling the in-order PE queue.
  - The four [4, 512] W2 matmuls of each 2048-row group are packed into
    the four 32-col strips of the PE array (tile_position col tiling):
    they stream their h operands concurrently on separate XBUSes and
    land in ONE PSUM bank at partition offsets 0/32/64/96 — ~4x less PE
    time than serial W2 matmuls.
  - PSUM: one shared 6-bank ring for the L1 accumulators (two live
    pairs = 4 banks + 2 slack) + a 2-bank ring for the packed L2 out.
  - Output stores are batched per group ([4, 2048] f32) and issued from
    the otherwise-idle gpsimd queue, so the ACT sequencer runs ReLUs
    back-to-back instead of interleaving ~667 ns DMA configs between
    them, and the tiny stores never block the V stream's buffer ring.
  - V-group DMAs (2 MiB, 16 KiB per-partition lines) stay on the sync
    queue with an 8-deep prefetch ring; the first group's DMA is split
    fine-grained and issued before anything else so the HBM stream
    starts immediately; groups 1-3 and the last group split per-chunk
    (PE runs close to the stream early on, and trailing compute
    overlaps the stream tail).
  - 38 small warm-up matmuls on a zeroed tile run during the initial
    DMA latency so the PE reaches full clock (HAM K=8/8) with margin
    before the first real matmul, and the p-state ramp is paid on
    dummies; every real matmul then runs at the full streaming rate.
  - The last group's epilogue is split in half across DVE/ACT for the
    b2-adds and gpsimd/scalar queues for the stores, shortening the
    drain tail.

Precision modes (KERNEL_MODE env var):
  f32    — plain fp32 matmuls (4x PE cycles, 4x DMA bytes).
  bf16   — single-pass bf16 (~3e-3 rel err).
  f16    — single-pass fp16 (~4e-4 rel err).
  f8     — V in fp8 e3m4, weights/h in f16 (~1.4e-2 rel err; default).
"""

import os
import sys

import numpy as np

for _p in ("/opt/trn_rl_repo", "/root/.axon_site/_ro/trn_rl_repo"):
    if os.path.isdir(_p) and _p not in sys.path:
        sys.path.insert(0, _p)

import concourse.bass as bass
import concourse.mybir as mybir
import concourse.tile as tile
from concourse import bacc
from concourse.bass_utils import run_bass_kernel_spmd

NCORES = 8
NN = 262144
IN_DIM = 1024
HIDDEN = 128
OUT_DIM = 4
R = NN // NCORES  # rows per core

P = 128           # SBUF partitions
KC = IN_DIM // P  # 8 k-chunks
CHUNK = 512       # rows per PSUM accumulation tile (one PSUM bank)
PAIR = 2 * CHUNK  # rows per L1 software-pipeline stage
GROUP = 2048      # rows per DMA group / per batched output store
DATA_BUFS = 8     # prefetch depth for V-group tiles

MODE = os.environ.get("KERNEL_MODE", "f8")

_last_results = None  # exposed for test harness (exec_time_ns etc.)


def _moving_dtype(mode):
    """dtype V streams in. f8 = fp8 e3m4 (4 mantissa bits): halves the
    HBM traffic; the PE upconverts at full precision at 1 col/cycle
    (single-rate; double-pumping would truncate to e6m3). Weights and h
    stay f16 — mixed-dtype matmul is allowed for non-fp32 operands."""
    if mode == "bf16":
        return mybir.dt.bfloat16
    if mode == "f16":
        return mybir.dt.float16
    if mode == "f8":
        return mybir.dt.float8e3
    return mybir.dt.float32


def _weight_dtype(mode):
    if mode == "f8":
        return mybir.dt.float16
    return _moving_dtype(mode)


def build_nc(mode=MODE, rows=R):
    """Build the SPMD Bass program for one core."""
    f32 = mybir.dt.float32
    mdt = _moving_dtype(mode)
    wdt = _weight_dtype(mode)

    nc = bacc.Bacc("TRN2")

    # V arrives host-prepacked per-group-contiguous [g, p, c, n]: a group
    # DMA is 128 descriptors x 16 KB of fully sequential HBM reads.
    ngroups_ = rows // GROUP
    vth_d = nc.declare_dram_parameter(
        "VTH", [ngroups_, P, KC, GROUP], mdt, isOutput=False
    )
    # W1 arrives host-prepacked in SBUF layout [P, KC*HIDDEN] so its DMA
    # moves 2 KB contiguous lines (128 descriptors) instead of 256 B ones.
    w1_d = nc.declare_dram_parameter("W1P", [P, KC * HIDDEN], wdt, isOutput=False)
    b1_d = nc.declare_dram_parameter("B1", [HIDDEN, 1], f32, isOutput=False)
    w2_d = nc.declare_dram_parameter("W2F", [HIDDEN, OUT_DIM], wdt, isOutput=False)
    b2_d = nc.declare_dram_parameter("B2", [OUT_DIM, 1], f32, isOutput=False)
    out_d = nc.declare_dram_parameter("OUT", [OUT_DIM, rows], f32, isOutput=True)

    ngroups = rows // GROUP
    nchunk = GROUP // CHUNK
    npairs = rows // PAIR
    pairs_per_group = GROUP // PAIR

    with tile.TileContext(nc) as tc:
        with (
            tc.tile_pool(name="const", bufs=1) as cpool,
            tc.tile_pool(name="data", bufs=DATA_BUFS) as dpool,
            tc.tile_pool(name="hbuf", bufs=6) as hpool,
            tc.tile_pool(name="obuf", bufs=6) as obpool,
            tc.tile_pool(name="psum1", bufs=6, space="PSUM") as ppool,
            tc.tile_pool(name="psum2", bufs=2, space="PSUM") as opool,
        ):
            vth_view = vth_d[:]
            outg_view = out_d[:].rearrange("o (g n) -> g o n", n=GROUP)

            # Bootstrap: put the first V chunk's DMA at the head of the
            # sync ring so the HBM stream starts immediately; weights
            # load on the scalar ring in parallel.
            vth0 = dpool.tile([P, KC, GROUP], mdt, tag="vth")
            nc.sync.dma_start(vth0[:, :, 0:CHUNK], vth_view[0][:, :, 0:CHUNK])

            w1_sb = cpool.tile([P, KC, HIDDEN], wdt)
            nc.sync.dma_start(
                w1_sb[:], w1_d[:].rearrange("p (c h) -> p c h", c=KC)
            )
            b1_sb = cpool.tile([HIDDEN, 1], f32)
            nc.scalar.dma_start(b1_sb[:], b1_d[:])
            w2_sb = cpool.tile([HIDDEN, OUT_DIM], wdt)
            nc.scalar.dma_start(w2_sb[:], w2_d[:])
            b2_sb = cpool.tile([OUT_DIM, 1], f32)
            nc.scalar.dma_start(b2_sb[:], b2_d[:])

            for u in range(1, 4):
                slu = slice(u * CHUNK, (u + 1) * CHUNK)
                nc.sync.dma_start(vth0[:, :, slu], vth_view[0][:, :, slu])

            # Warm-up: small dummy matmuls on a zeroed [128, 128] tile
            # keep the PE busy during the initial DMA latency so it
            # reaches full clock (HAM K=8/8) with margin before the first
            # real matmul arrives; the p-state ramp is paid on dummies.
            zz = cpool.tile([P, HIDDEN], mdt, tag="zz")
            nc.vector.memset(zz[:], 0.0)
            ps_warm = ppool.tile([HIDDEN, CHUNK], f32, tag="ps")
            for _ in range(38):
                nc.tensor.matmul(ps_warm[:, 0:HIDDEN], zz[:], zz[:],
                                 start=True, stop=True)

            state = {"h": []}

            def relu_pair(pp, ps0, ps1):
                # Deferred ReLU for pair pp: by the time this runs, pair
                # pp's PSUM has been drainable for a full pair of L1
                # matmuls, so the ACT engine is never on the PE's
                # critical path.
                for ps in (ps0, ps1):
                    h_sb = hpool.tile([HIDDEN, CHUNK], wdt, tag="h")
                    nc.scalar.activation(
                        h_sb[:], ps[:],
                        mybir.ActivationFunctionType.Relu,
                        bias=b1_sb[:],
                    )
                    state["h"].append(h_sb)

            def l2_half_flush(q, half, hs2, store_engine):
                # Half-group flush for the tail: two col-tiled W2 matmuls
                # into one PSUM bank, b2-adds split across DVE and ACT,
                # then a [4, 2*CHUNK] store so the drain chain of the
                # last pairs runs in parallel across engines/queues.
                po = opool.tile([P, CHUNK], f32, tag="po")
                for j, h_sb in enumerate(hs2):
                    nc.tensor.matmul(
                        po[32 * j:32 * j + OUT_DIM, :], w2_sb[:], h_sb[:],
                        start=True, stop=True, tile_position=(0, 32 * j),
                    )
                o_sb = obpool.tile([OUT_DIM, 2 * CHUNK], f32, tag="oh", bufs=2)
                nc.vector.tensor_scalar_add(
                    o_sb[:, 0:CHUNK], po[0:OUT_DIM, :], b2_sb[:]
                )
                nc.scalar.activation(
                    o_sb[:, CHUNK:], po[32:32 + OUT_DIM, :],
                    mybir.ActivationFunctionType.Identity,
                    bias=b2_sb[:],
                )
                half_view = out_d[:].rearrange("o (m n) -> m o n", n=2 * CHUNK)
                store_engine.dma_start(half_view[2 * q + half], o_sb[:])

            def l2_flush(q):
                # Second layer for group q: the group's four [4, CHUNK]
                # W2-matmuls are packed into the four 32-col strips of
                # the PE array (tile_position col tiling) so they stream
                # their four h operands concurrently on separate XBUSes,
                # all landing in one PSUM bank at partition offsets
                # 0/32/64/96.
                hs = state["h"]
                assert len(hs) == nchunk
                po = opool.tile([P, CHUNK], f32, tag="po")
                for j in range(nchunk):
                    nc.tensor.matmul(
                        po[32 * j:32 * j + OUT_DIM, :], w2_sb[:], hs[j][:],
                        start=True, stop=True, tile_position=(0, 32 * j),
                    )
                state["h"] = []
                o_tile = obpool.tile([OUT_DIM, GROUP], f32, tag="o")
                for j in range(nchunk):
                    nc.vector.tensor_scalar_add(
                        o_tile[:, j * CHUNK:(j + 1) * CHUNK],
                        po[32 * j:32 * j + OUT_DIM, :], b2_sb[:]
                    )
                nc.gpsimd.dma_start(outg_view[q], o_tile[:])

            prev = None
            for g in range(ngroups):
                if g == 0:
                    vth = vth0
                else:
                    vth = dpool.tile([P, KC, GROUP], mdt, tag="vth")
                    if g in (1, 2, 3) or g == ngroups - 1:
                        # Split early groups (PE is close on the stream's
                        # heels while prefetch builds) and the last group
                        # (trailing compute overlaps the stream tail) so
                        # data lands at finer granularity.
                        for u in range(nchunk):
                            slu = slice(u * CHUNK, (u + 1) * CHUNK)
                            nc.sync.dma_start(vth[:, :, slu], vth_view[g][:, :, slu])
                    else:
                        nc.sync.dma_start(vth[:], vth_view[g])

                for half in range(pairs_per_group):
                    pp = g * pairs_per_group + half
                    sl0 = slice((2 * half) * CHUNK, (2 * half + 1) * CHUNK)
                    sl1 = slice((2 * half + 1) * CHUNK, (2 * half + 2) * CHUNK)

                    ps0 = ppool.tile([HIDDEN, CHUNK], f32, tag="ps")
                    ps1 = ppool.tile([HIDDEN, CHUNK], f32, tag="ps")
                    if pp == npairs - 1:
                        # Final pair, split emission: the penultimate
                        # half-group's flush and this pair's first ReLU
                        # are sandwiched between the two accumulator
                        # halves so they execute during the second
                        # half's matmuls — only one ReLU, the last
                        # W2 pack, two adds and one store remain in the
                        # drain tail after the very last L1 matmul.
                        for c in range(KC):
                            nc.tensor.matmul(
                                ps0[:], w1_sb[:, c, :], vth[:, c, sl0],
                                start=(c == 0), stop=(c == KC - 1),
                            )
                        relu_pair(*prev)
                        h62 = hpool.tile([HIDDEN, CHUNK], wdt, tag="h")
                        nc.scalar.activation(
                            h62[:], ps0[:],
                            mybir.ActivationFunctionType.Relu, bias=b1_sb[:]
                        )
                        for c in range(KC):
                            nc.tensor.matmul(
                                ps1[:], w1_sb[:, c, :], vth[:, c, sl1],
                                start=(c == 0), stop=(c == KC - 1),
                            )
                        l2_half_flush(ngroups - 1, 0, state["h"], nc.gpsimd)
                        state["h"] = []
                        prev = (pp, ps0, ps1)
                        continue
                    if g in (1, 2):
                        # Early groups run right at the stream's edge:
                        # consume the pair's two chunks sequentially so
                        # the second chunk's DMA gets 8 matmuls of slack.
                        for ps, sl in ((ps0, sl0), (ps1, sl1)):
                            for c in range(KC):
                                nc.tensor.matmul(
                                    ps[:], w1_sb[:, c, :], vth[:, c, sl],
                                    start=(c == 0), stop=(c == KC - 1),
                                )
                    else:
                        # Interleave the two accumulators so each W1
                        # k-chunk weight load serves two matmuls.
                        for c in range(KC):
                            nc.tensor.matmul(
                                ps0[:], w1_sb[:, c, :], vth[:, c, sl0],
                                start=(c == 0), stop=(c == KC - 1),
                            )
                            nc.tensor.matmul(
                                ps1[:], w1_sb[:, c, :], vth[:, c, sl1],
                                start=(c == 0), stop=(c == KC - 1),
                            )

                    if prev is not None:
                        relu_pair(*prev)
                        if prev[0] % pairs_per_group == pairs_per_group - 1:
                            l2_flush(prev[0] // pairs_per_group)
                    prev = (pp, ps0, ps1)

            # Drain tail: the final chunk's ReLU on DVE (ACT is busy with
            # the half-A add and the store-B config), then the last pack.
            _, ps0, ps1 = prev
            h63 = hpool.tile([HIDDEN, CHUNK], wdt, tag="h")
            nc.vector.tensor_scalar(
                h63[:], ps1[:], b1_sb[:], 0.0,
                mybir.AluOpType.add, mybir.AluOpType.max,
            )
            l2_half_flush(ngroups - 1, 1, [h62, h63], nc.scalar)

    return nc


def kernel(V, W1, b1, W2, b2):
    global _last_results
    mode = MODE
    mdt = _moving_dtype(mode)

    def _np_dt(bdt):
        if bdt == mybir.dt.float32:
            return np.float32
        if bdt == mybir.dt.float16:
            return np.float16
        import ml_dtypes

        if bdt == mybir.dt.bfloat16:
            return ml_dtypes.bfloat16
        return ml_dtypes.float8_e3m4

    np_vdt = _np_dt(mdt)
    np_wdt = _np_dt(_weight_dtype(mode))

    V = np.asarray(V, dtype=np.float32)
    W1 = np.asarray(W1, dtype=np.float32)
    b1 = np.asarray(b1, dtype=np.float32)
    W2 = np.asarray(W2, dtype=np.float32)
    b2 = np.asarray(b2, dtype=np.float32)

    # Prepack W1 into the SBUF tile layout [P, KC*HIDDEN]:
    # element (c*P + p, h) of W1 lands at [p, c*HIDDEN + h].
    w1p = np.ascontiguousarray(
        W1.astype(np_wdt).reshape(KC, P, HIDDEN).transpose(1, 0, 2).reshape(P, KC * HIDDEN)
    )
    common = {
        "W1P": w1p,
        "B1": np.ascontiguousarray(b1.reshape(HIDDEN, 1)),
        "W2F": np.ascontiguousarray(W2.astype(np_wdt)),
        "B2": np.ascontiguousarray(b2.reshape(OUT_DIM, 1)),
    }

    in_maps = []
    for c in range(NCORES):
        shard = V[c * R : (c + 1) * R]  # [R, IN_DIM]
        q = shard.astype(np_vdt)  # [R, IN_DIM] quantized
        m = {"VTH": np.ascontiguousarray(
            q.reshape(R // GROUP, GROUP, KC, P).transpose(0, 3, 2, 1)
        )}
        m.update(common)
        in_maps.append(m)

    nc = build_nc(mode, R)
    nc.finalize()
    res = run_bass_kernel_spmd(nc, in_maps, list(range(NCORES)))
    _last_results = res

    out = np.concatenate(
        [np.asarray(r["OUT"]).T for r in res.results], axis=0
    ).astype(np.float32)
    return out
